# revision 27
# baseline (speedup 1.0000x reference)
"""CosFace loss (B=1024, D=512, C=100000) on 8 Trainium2 NeuronCores.

Strategy (tensor-parallel classification head, per sharding hint):
  - Classes sharded 12500/core (padded to 12544 = 98*128 with zero rows).
  - fp8 (e4m3) matmul in DoubleRow perf mode (0.5 cycles/out-row): the PE
    produces cosine tiles [128b, 2048c] in PSUM in ~0.85us each; total PE
    busy ~44us/core.  The kernel bottleneck is draining those 12.8M PSUM
    f32 values through exp+sum, so that work is SPLIT between the two
    engines that can read PSUM:
      * ScalarE (ACT): exp(scl_b*psum - 64) with accum_out on cols
        [0, ACOLS) — 1 elem/cycle/lane at 1.2 GHz.
      * VectorE (DVE): cols [ACOLS, 2048) via the exp bit trick:
        i16 = round(psum*(scl_b*log2e*2^7) + (127-C)*2^7) is exactly the
        bit pattern of bf16(e^z) (C=0.0573 zeros the mean frac error);
        a second pass (scalar_tensor_tensor identity + accum_out) sums
        the bf16 values into f32 per-row partials.  Two passes at
        1 elem/cycle/lane at 0.96 GHz.
    ACOLS balances the two pipelines.
  - No running max is needed: |logits| <= S = 64 bounds every exponent
    inside f32/bf16 range (ACT lane shifts by -64; DVE lane uses the
    bf16 exponent field directly, values in [2^-97, 2^97]).
  - Each core ships per-row partial sum-of-exp vectors for both lanes;
    the host sums the 8 cores (the gather/unshard step), rescales the
    DVE lane by e^-64, and finishes the scalar loss with the exact
    label-term swap (dneg) as in the baseline.
"""

import numpy as np

import concourse.bass as bass
import concourse.mybir as mybir
import concourse.tile as tile
from concourse import bacc
from concourse.bass_utils import run_bass_kernel_spmd

B, D, C = 1024, 512, 100000
S, MARGIN = 64.0, 0.35
SHIFT = 64.0
NCORES = 8
CSHARD = C // NCORES          # 12500 real classes per core
CLOC = 12544                  # padded (98 * 128)
KT2 = D // 256                # 2 double-row contraction tiles
MT = B // 128                 # 8 batch tiles
SCALE_X = 8.0
SCALE_W = 64.0
import os as _os
CHUNKA = 2048                 # PSUM tile allocation (4 banks; 2 bufs = all 8)
# first/last chunks small so the drain pipeline fills fast and flushes fast
CHUNKS = [512, 2048, 2048, 2048, 2048, 2048, 1408, 384]   # sums to CLOC
NCHUNK = len(CHUNKS)
# DVE-lane columns per chunk size (ScalarE takes the rest); last chunk
# all-ScalarE so the tail drains both engines concurrently.  GpSimd
# compacts each DVE tile by pairwise-add before the accum pass, so the
# DVE-lane widths are kept even.
WSPLIT = {512: 176, 2048: 656, 1408: 448, 384: 144}
NMM = 512                     # out columns per matmul (moving free = 1024 fp8)
WBUFS = 3
PBUFS = 2
WCMAX = 768                   # max DVE cols per tile (ibuf allocation)
CMAGIC = 0.0573
LOG2E = float(np.log2(np.e))
B16C = float((127.0 - CMAGIC) * 2.0 ** 7)

F32 = mybir.dt.float32
F8 = mybir.dt.float8e4
I16 = mybir.dt.int16
BF16 = mybir.dt.bfloat16
AF = mybir.ActivationFunctionType
AX = mybir.AxisListType
ALU = mybir.AluOpType
DR = mybir.MatmulPerfMode.DoubleRow

_NC = None
LAST_RESULTS = None
ABLATE = 'full'  # full | noexp | nomm | nodma


def _body(nc, tc, xt, wt, scl, sclb, loss, dvep):
    from contextlib import ExitStack
    with ExitStack() as ctx:
        singles = ctx.enter_context(tc.tile_pool(name="singles", bufs=1))
        wpool = ctx.enter_context(tc.tile_pool(name="wpool", bufs=WBUFS))
        ipool = ctx.enter_context(tc.tile_pool(name="ipool", bufs=4))
        hpool = ctx.enter_context(tc.tile_pool(name="hpool", bufs=3))
        jpool = ctx.enter_context(tc.tile_pool(name="jpool", bufs=2))
        psump = ctx.enter_context(tc.tile_pool(name="psump", bufs=PBUFS, space="PSUM"))
        wt_v = wt.ap().rearrange("(k i p) c -> p k i c", p=128, i=2)
        wt_cs = []
        for c in range(NCHUNK):
            wt_chunk_tile = wpool.tile([128, KT2, 2, CHUNKA], F8, tag="wt")
            wt_cs.append(wt_chunk_tile)
        xt_sb = singles.tile([128, KT2, 2, B], F8)
        xt_v = xt.ap().rearrange("(k i p) b -> p k i b", p=128, i=2)
        # small vectors first on the gpsimd DGE queue, then x rides the
        # same queue while the first weight chunk goes on the sync queue
        scl_sb = singles.tile([128, MT], F32)
        nc.gpsimd.dma_start(out=scl_sb[:, :], in_=scl.ap()[:, :])
        sclb_sb = singles.tile([128, MT], F32)
        nc.gpsimd.dma_start(out=sclb_sb[:, :], in_=sclb.ap()[:, :])
        if ABLATE != 'nodma':
            nc.sync.dma_start(out=xt_sb[:, 0:1, :, :], in_=xt_v[:, 0:1, :, :])
            nc.sync.dma_start(out=wt_cs[0][:, :, :, 0:CHUNKS[0]],
                              in_=wt_v[:, :, :, 0:CHUNKS[0]])
            nc.sync.dma_start(out=xt_sb[:, 1:2, :, :], in_=xt_v[:, 1:2, :, :])
        else:
            nc.sync.dma_start(out=xt_sb[:, :, :, :], in_=xt_v[:, :, :, :])

        # const bias column (activation bias must be a [P,1] AP)
        cb_m64 = singles.tile([128, 1], F32)
        nc.vector.memset(cb_m64[:, :], -SHIFT)
        # dummy activation on ready data hoists the 1.3us Exp table load to
        # t~0 (it would otherwise wait for the first PSUM tile)
        tbl_warm = singles.tile([128, 1], F32)
        nc.scalar.activation(tbl_warm[:, :], cb_m64[:, :], AF.Exp,
                             bias=cb_m64[:, :])

        # ---- main loop: cosine matmuls + fused exp/accumulate ----
        sums_a = singles.tile([128, MT, NCHUNK], F32)
        nc.vector.memset(sums_a[:, :, :], 0.0)
        sums_d = singles.tile([128, MT, NCHUNK], F32)
        nc.vector.memset(sums_d[:, :, :], 0.0)
        coff = 0
        for c in range(NCHUNK):
            c0 = coff
            ncls = CHUNKS[c]
            coff += ncls
            wcols = WSPLIT[ncls] if ABLATE != 'noexp' else 0
            acols = ncls - wcols
            wt_c = wt_cs[c]
            if ABLATE != 'nodma' and c > 0:
                nc.sync.dma_start(out=wt_c[:, :, :, :ncls],
                                  in_=wt_v[:, :, :, c0:c0 + ncls])
            for m in range(MT):
                g = psump.tile([128, CHUNKA], F32, tag="g")
                if ABLATE != 'nomm':
                    for k in range(KT2):
                        lhsT = xt_sb[:, k, :, m * 128:(m + 1) * 128]
                        for n in range(0, ncls, NMM):
                            nsz = min(NMM, ncls - n)
                            nc.tensor.matmul(g[:, n:n + nsz], lhsT,
                                             wt_c[:, k, :, n:n + nsz],
                                             start=(k == 0), stop=(k == KT2 - 1),
                                             perf_mode=DR)
                if ABLATE != 'noexp':
                    # ScalarE lane: in-place exp on PSUM, accum -> row partial
                    nc.scalar.activation(g[:, :acols], g[:, :acols], AF.Exp,
                                         bias=cb_m64[:, :],
                                         scale=scl_sb[:, m:m + 1],
                                         accum_out=sums_a[:, m, c:c + 1])
                    if wcols > 0:
                        # DVE lane: bf16 bit-trick exp, then identity+accum
                        ib = ipool.tile([128, WCMAX], I16, tag="ib")
                        nc.vector.tensor_scalar(
                            ib[:, :wcols], g[:, acols:ncls],
                            sclb_sb[:, m:m + 1], B16C, ALU.mult, ALU.add)
                        bv = ib[:, :wcols].bitcast(BF16)
                        jb = jpool.tile([128, WCMAX], BF16, tag="jb")
                        nc.vector.scalar_tensor_tensor(
                            jb[:, :wcols], bv, 1.0, bv, ALU.mult, ALU.max,
                            accum_out=sums_d[:, m, c:c + 1])

        # ---- reduce partials; host sums cores and finishes the loss ----
        se_a = singles.tile([128, MT], F32)
        nc.vector.tensor_reduce(se_a[:, :], sums_a[:, :, :], axis=AX.X, op=ALU.add)
        nc.sync.dma_start(out=loss.ap()[:, :], in_=se_a[:, :])
        se_d = singles.tile([128, MT], F32)
        nc.vector.tensor_reduce(se_d[:, :], sums_d[:, :, :], axis=AX.X, op=ALU.add)
        nc.sync.dma_start(out=dvep.ap()[:, :], in_=se_d[:, :])


def _build(repeat=1):
    nc = bacc.Bacc("TRN2", target_bir_lowering=False, debug=False,
                   num_devices=NCORES)
    xt = nc.dram_tensor("xt", [D, B], F8, kind="ExternalInput")
    wt = nc.dram_tensor("wt", [D, CLOC], F8, kind="ExternalInput")
    scl = nc.dram_tensor("scl", [128, MT], F32, kind="ExternalInput")
    sclb = nc.dram_tensor("sclb", [128, MT], F32, kind="ExternalInput")
    loss = nc.dram_tensor("loss", [128, MT], F32, kind="ExternalOutput")
    dvep = nc.dram_tensor("dvep", [128, MT], F32, kind="ExternalOutput")
    with tile.TileContext(nc) as tc:
        for _ in range(repeat):
            _body(nc, tc, xt, wt, scl, sclb, loss, dvep)
    nc.compile()
    return nc


def _get_nc():
    global _NC
    if _NC is None:
        _NC = _build()
    return _NC


def _to_pcol(v):
    """[B] -> [128, MT] with b = m*128 + p at [p, m]."""
    return np.ascontiguousarray(np.asarray(v, dtype=np.float32).reshape(MT, 128).T)


def _prep(inputs):
    f8 = mybir.dt.np(F8)
    x = np.asarray(inputs["input"], dtype=np.float64)
    w = np.asarray(inputs["weight"], dtype=np.float64)
    wn = w / np.maximum(np.sqrt((w * w).sum(axis=1, keepdims=True)), 1e-12)
    xnorm = np.maximum(np.sqrt((x * x).sum(axis=1)), 1e-12)
    scl = S / (SCALE_X * SCALE_W * xnorm)
    xt8 = np.clip(x.T * SCALE_X, -224.0, 224.0).astype(f8)
    xt8 = np.ascontiguousarray(xt8)
    scl_c = _to_pcol(scl)
    sclb_c = _to_pcol(scl * LOG2E * 2.0 ** 7)
    in_maps = []
    for k in range(NCORES):
        shard = np.zeros((D, CLOC), dtype=f8)
        blk = np.clip(wn[k * CSHARD:(k + 1) * CSHARD].T * SCALE_W, -224.0, 224.0)
        shard[:, :CSHARD] = blk.astype(f8)
        in_maps.append({"xt": xt8, "wt": shard, "scl": scl_c, "sclb": sclb_c})
    return in_maps


def _host_tail(inputs, partials_a, partials_d):
    """partials: per-core [128, MT] sum-of-exp row partials (two lanes)."""
    x = np.asarray(inputs["input"], dtype=np.float64)
    label = np.asarray(inputs["label"]).astype(np.int64)
    w = np.asarray(inputs["weight"], dtype=np.float64)
    wl = w[label]
    wln = wl / np.maximum(np.sqrt((wl * wl).sum(axis=1, keepdims=True)), 1e-12)
    xnorm = np.maximum(np.sqrt((x * x).sum(axis=1)), 1e-12)
    cos_l = (x * wln).sum(axis=1) / xnorm
    se = np.zeros(B, dtype=np.float64)
    dsc = np.exp(-SHIFT)
    for pa, pd in zip(partials_a, partials_d):
        se += np.asarray(pa, dtype=np.float64).T.reshape(B)
        se += dsc * np.asarray(pd, dtype=np.float64).T.reshape(B)
    dneg = np.exp(S * cos_l - SHIFT) - np.exp(S * cos_l - S * MARGIN - SHIFT)
    lossrow = np.log(se - dneg) + SHIFT - (S * cos_l - S * MARGIN)
    return np.float32(lossrow.mean())


def kernel(**inputs):
    global LAST_RESULTS
    # this axon client build has no NTFF hook; a stray BASS_TRACE=1 in the
    # environment would crash run_bass_kernel_spmd on an optional import
    _os.environ["BASS_NEVER_TRACE"] = "1"
    nc = _get_nc()
    in_maps = _prep(inputs)
    res = run_bass_kernel_spmd(nc, in_maps, core_ids=list(range(NCORES)))
    LAST_RESULTS = res
    pa = [res.results[k]["loss"] for k in range(NCORES)]
    pd = [res.results[k]["dvep"] for k in range(NCORES)]
    return np.asarray(_host_tail(inputs, pa, pd), dtype=np.float32)
